# revision 77
# baseline (speedup 1.0000x reference)
"""GQA attention kernel for Trainium2, 8 NeuronCores.

Sharding: data-parallel over batch (B=2) x 4 head-shards -> 8 cores.
Shard s owns q-heads {2s, 2s+1, 2s+8, 2s+9}; heads h and h+8 are
rotate-half RoPE partners, so each shard's RoPE is self-contained.
Those 4 heads use kv-heads {s//2, s//2+2}, which are RoPE partners on
the K side.  out_proj is row-parallel; partials are summed on host.

v3 layout (cost-model driven):
  front: per-(so,ko) input DMAs ordered so PE starts ~2us in; K
    projection packed to 128 output partitions; rope via same-base
    DVE ops with output partition shift; rmsnorm via block-ones
    matmul + ACT Sqrt + DVE bf16 reciprocal; squares on Pool;
    kn_swapped built by base-shifted normalize ops (DVE/Pool).
  transition: V projections and the last rms chain ride in the
    attention PSUM pool's tag rotation as PE filler while the first
    scores steps issue - no dead zone between phases.
  attention: flat pipeline over 128 (ic, jc) steps; exp split
    ACT(10)/DVE-Schraudolph(6) per 16 jc (gpsimd cannot read PSUM);
    pss triple-buffered so scores(g+3) never waits on exp(g); PV with
    ones-column denominator; single fused normalize per ic; out_proj
    matmuls for ic-1 interleaved at jc 6/9/12/15 (psy lives in the
    pss tag rotation); oT via XBAR dma transpose; y copies on DVE.
"""

import numpy as np
import ml_dtypes

import concourse.bacc as bacc
import concourse.mybir as mybir
from concourse.tile import TileContext
from concourse.bass_utils import run_bass_kernel_spmd

BF16 = mybir.dt.bfloat16
F32 = mybir.dt.float32
I16 = mybir.dt.int16
AL = mybir.AluOpType
AF = mybir.ActivationFunctionType

# Schraudolph fast-exp in bf16 bit-space: exp(s/8) ~= bitcast_bf16(
# int16(SCH_A*s + SCH_B)).  RMS rel err ~1.8% on N(0,1) scores.
SCH_A = (2.0 ** 7 / np.log(2.0)) * 0.125
SCH_B = 127.0 * 2.0 ** 7 - 7.375

B, S, HID = 2, 2048, 1024
H, HKV, D = 16, 4, 64
ROPE_BASE = 10000.0
EPS = float(np.finfo(np.float32).eps)
NB = ml_dtypes.bfloat16

NIC = 8          # i-chunks of 256
ICS = S // NIC   # 256
NJC = 16         # j-chunks of 128
LK = 3           # PV lookahead in jc-steps

# per-jc exp engine within each i-chunk: ACT (exact) for most, DVE
# Schraudolph for the rest (Pool/gpsimd cannot read PSUM on TRN2).
# DVE steps sit away from the ic boundary (jc 14-15, 0) so the finish
# chain (reciprocal+normalize, also DVE) is not queued behind an exp.
JC_ENG = ["act"] * NJC
for _j in (3, 5, 7, 9, 11, 13):
    JC_ENG[_j] = "dve"

_cache: dict = {}


def _build(use_mask: bool, use_bias: bool):
    nc = bacc.Bacc("TRN2", target_bir_lowering=False)

    hT = nc.dram_tensor("hT", [128, 8, S], BF16, kind="ExternalInput")
    wq = nc.dram_tensor("wq", [128, 8, 256], BF16, kind="ExternalInput")
    wkv = nc.dram_tensor("wkv", [128, 8, 256], BF16, kind="ExternalInput")
    wo = nc.dram_tensor("wo", [128, 2, HID], BF16, kind="ExternalInput")
    # qtab: [cos, -sin, +sin]; ktab: [cos-dup, crossed-sign sin]
    qtab = nc.dram_tensor("qtab", [128, 3, S], BF16, kind="ExternalInput")
    ktab = nc.dram_tensor("ktab", [128, 2, S], BF16, kind="ExternalInput")
    ident = nc.dram_tensor("ident", [128, 128], BF16, kind="ExternalInput")
    if use_bias:
        bias = nc.dram_tensor("bias", [1, 512], F32, kind="ExternalInput")
    mk = (
        nc.dram_tensor("mk", [NJC, 128, S], F32, kind="ExternalInput")
        if use_mask
        else None
    )
    y = nc.dram_tensor("y", [128, NIC, 2, HID], BF16, kind="ExternalOutput")

    with TileContext(nc) as tc:
        with tc.tile_pool(name="const", bufs=1) as cp:
            # ---- persistent SBUF tiles --------------------------------
            hT_sb = cp.tile([128, 8, S], BF16)
            wq_sb = cp.tile([128, 8, 256], BF16)
            wkv_sb = cp.tile([128, 8, 256], BF16)  # [:, :, 0:128]=k, 128:256=v
            wo_sb = cp.tile([128, 2, HID], BF16)
            qco_sb = cp.tile([128, S], BF16)
            qsi2_sb = cp.tile([128, 2, S], BF16)  # [-sin, +sin]
            kco_sb = cp.tile([128, S], BF16)   # cos, duplicated halves
            ksi_sb = cp.tile([128, S], BF16)   # sin, crossed signs

            qn = cp.tile([128, 2, S], BF16)    # rmsnorm'd roped q
            kn = cp.tile([128, S], BF16)       # packed k: [kva d | kvb d]
            kn_sw = cp.tile([128, S], BF16)    # half-swapped copy
            v_all = cp.tile([128, NJC, 2, 65], BF16)  # v natural + ones col
            y_sb = cp.tile([128, NIC, 2, HID], BF16)
            oT = cp.tile([128, 2, NIC, ICS], BF16)  # [d-part, cc, ic, i]

            ident_sb = cp.tile([128, 128], BF16)
            eps_sb = cp.tile([128, 1], F32)
            nc.vector.memset(eps_sb[:], EPS)
            onesq = cp.tile([128, 128], BF16)  # block-diag 1/64
            nc.vector.memset(onesq[:], 0.0)
            nc.vector.memset(onesq[0:64, 0:64], 1.0 / 64.0)
            nc.vector.memset(onesq[64:128, 64:128], 1.0 / 64.0)
            nc.vector.memset(v_all[:], 1.0)
            if use_bias:
                ones_row = cp.tile([1, 512], BF16)
                nc.vector.memset(ones_row[:], 1.0)
                bias_sb = cp.tile([1, 512], F32)

            # ---- input DMAs: fine-grained, ordered for early PE start --
            # hT arrives per (so, ko); the previous quarter's rope tables
            # interleave one-per-hT-chunk so neither stream starves; wo
            # (needed only at out_proj) goes last.
            # HWDGE holds ~625ns per DMA, so few/large DMAs: hT per
            # (ko, S-half), rope tables per S-half, wq in two pieces.
            def tab_dmas(h):
                sl = slice(h * 1024, (h + 1) * 1024)
                nc.sync.dma_start(out=qco_sb[:, sl], in_=qtab[:, 0, sl])
                nc.sync.dma_start(out=qsi2_sb[:, :, sl], in_=qtab[:, 1:3, sl])
                nc.sync.dma_start(out=kco_sb[:, sl], in_=ktab[:, 0, sl])
                nc.sync.dma_start(out=ksi_sb[:, sl], in_=ktab[:, 1, sl])

            nc.sync.dma_start(out=wq_sb[:, 0, :], in_=wq[:, 0, :])
            nc.sync.dma_start(out=hT_sb[:, 0, 0:1024], in_=hT[:, 0, 0:1024])
            nc.sync.dma_start(out=wq_sb[:, 1:8, :], in_=wq[:, 1:8, :])
            nc.sync.dma_start(out=hT_sb[:, 1, 0:1024], in_=hT[:, 1, 0:1024])
            nc.sync.dma_start(out=wkv_sb[:], in_=wkv[:])
            if use_bias:
                nc.sync.dma_start(out=bias_sb[:], in_=bias[:])
            for ko in range(2, 8):
                nc.sync.dma_start(
                    out=hT_sb[:, ko, 0:1024], in_=hT[:, ko, 0:1024]
                )
            tab_dmas(0)
            for ko in range(8):
                nc.sync.dma_start(
                    out=hT_sb[:, ko, 1024:2048], in_=hT[:, ko, 1024:2048]
                )
            tab_dmas(1)
            nc.sync.dma_start(out=wo_sb[:], in_=wo[:])
            nc.sync.dma_start(out=ident_sb[:], in_=ident[:])

            chains = {}

            def psv_group(group, psv, one_bank):
                """V projection for 4 position-chunks into psv slices
                [:, i, 0:128]; accumulation-group flags per psum bank."""
                scs = range(4 * group, 4 * group + 4)
                for ko in range(8):
                    for i in range(4):
                        first = i == 0 or (not one_bank and i == 2)
                        last = i == 3 or (not one_bank and i == 1)
                        nc.tensor.matmul(
                            psv[:, i, 0:128],
                            lhsT=hT_sb[:, ko,
                                       scs[i] * 128:(scs[i] + 1) * 128],
                            rhs=wkv_sb[:, ko, 128:256],
                            start=(ko == 0 and first),
                            stop=(ko == 7 and last)
                            if not use_bias else False,
                        )
                if use_bias:
                    for i in range(4):
                        last = i == 3 or (not one_bank and i == 1)
                        nc.tensor.matmul(
                            psv[:, i, 0:128],
                            lhsT=ones_row[:, 0:128],
                            rhs=bias_sb[:, 384:512],
                            start=False, stop=last,
                        )
                for i in range(4):
                    nc.scalar.copy(
                        v_all[:, scs[i], :, 0:64],
                        psv[:, i, 0:128].rearrange("p (c d) -> p c d", c=2),
                    )

            def emit_rope(so, psq, psk, fsb):
                """ACT casts to bf16, DVE rope combines (inputs share a
                partition base; outputs shift), Pool squares."""
                sl = slice(so * 512, (so + 1) * 512)
                rawq = fsb.tile([128, 2, 512], BF16, tag="rawq", bufs=1,
                                name="rawq")
                nc.scalar.copy(rawq, psq)
                rawk = fsb.tile([128, 512], BF16, tag="rawk", bufs=1,
                                name="rawk")
                nc.scalar.copy(rawk, psk)

                uq = fsb.tile([128, 2, 512], BF16, tag="uq", bufs=1,
                              name="uq")
                nc.vector.tensor_tensor(
                    uq, rawq,
                    qco_sb[:, None, sl].to_broadcast((128, 2, 512)),
                    AL.mult,
                )
                # signs are folded into qsi2 ([-sin, +sin]) so one add
                # combines both rope halves
                wq_t = fsb.tile([128, 2, 512], BF16, tag="wq_t", bufs=1,
                                name="wq_t")
                nc.vector.tensor_tensor(
                    wq_t[:, 0, :], rawq[:, 1, :], qsi2_sb[:, 0, sl], AL.mult
                )
                nc.vector.tensor_tensor(
                    wq_t[:, 1, :], rawq[:, 0, :], qsi2_sb[:, 1, sl], AL.mult
                )
                rpq = cp.tile([128, 2, 512], BF16, name=f"rpq{so}")
                nc.vector.tensor_tensor(rpq, uq, wq_t, AL.add)
                sqq = cp.tile([128, 2, 512], BF16, name=f"sqq{so}")
                # so3's squares on DVE: its chain gates the attention start
                sq_eng = nc.vector if so == 3 else nc.gpsimd
                sq_eng.tensor_tensor(sqq, rpq, rpq, AL.mult)

                tk = fsb.tile([128, 512], BF16, tag="tk", bufs=1, name="tk")
                nc.vector.tensor_tensor(tk, rawk, kco_sb[:, sl], AL.mult)
                # ksi carries crossed signs (+sin on parts 0:64, -sin on
                # 64:128): both inputs share a partition base (HW rule),
                # only the output is base-shifted, and one add combines.
                wk_t = fsb.tile([128, 512], BF16, tag="wk_t", bufs=1,
                                name="wk_t")
                nc.vector.tensor_tensor(
                    wk_t[0:64, :], rawk[64:128, :], ksi_sb[64:128, sl],
                    AL.mult,
                )
                nc.vector.tensor_tensor(
                    wk_t[64:128, :], rawk[0:64, :], ksi_sb[0:64, sl],
                    AL.mult,
                )
                rpk = cp.tile([128, 512], BF16, name=f"rpk{so}")
                nc.vector.tensor_tensor(rpk, tk, wk_t, AL.add)
                sqk = cp.tile([128, 512], BF16, name=f"sqk{so}")
                sq_eng.tensor_tensor(sqk, rpk, rpk, AL.mult)
                chains[so] = (rpq, sqq, rpk, sqk)

            def emit_rms(so, psrq, psrk, fsb):
                """Sqrt on ACT, bf16 reciprocal on DVE, normalize multiplies
                on DVE/Pool; K side first (the last chunk's kn gates the
                attention start); kn_sw written with shifted output bases."""
                sl = slice(so * 512, (so + 1) * 512)
                rpq, sqq, rpk, sqk = chains.pop(so)
                k_first = so == 3
                if k_first:
                    nc.tensor.matmul(
                        psrk[:], lhsT=onesq[:], rhs=sqk[:],
                        start=True, stop=True,
                    )
                for ch in range(2):
                    nc.tensor.matmul(
                        psrq[:, ch, :], lhsT=onesq[:], rhs=sqq[:, ch, :],
                        start=True, stop=True,
                    )
                if not k_first:
                    nc.tensor.matmul(
                        psrk[:], lhsT=onesq[:], rhs=sqk[:],
                        start=True, stop=True,
                    )

                def k_side():
                    rsk32 = fsb.tile([128, 512], F32, tag="rsk32", bufs=1,
                                     name="rsk32")
                    nc.scalar.activation(rsk32, psrk, AF.Sqrt, bias=eps_sb[:])
                    rsk = fsb.tile([128, 512], BF16, tag="rsk", bufs=1,
                                   name="rsk")
                    with nc.allow_low_precision(reason="rms scale in bf16"):
                        nc.vector.reciprocal(rsk, rsk32)
                    nc.vector.tensor_tensor(kn[:, sl], rpk, rsk, AL.mult)
                    nc.vector.tensor_tensor(
                        kn_sw[0:64, sl], rpk[64:128, :], rsk[64:128, :],
                        AL.mult,
                    )
                    nc.gpsimd.tensor_tensor(
                        kn_sw[64:128, sl], rpk[0:64, :], rsk[0:64, :],
                        AL.mult,
                    )

                def q_side():
                    rsq32 = fsb.tile([128, 2, 512], F32, tag="rsq32", bufs=1,
                                     name="rsq32")
                    nc.scalar.activation(rsq32, psrq, AF.Sqrt, bias=eps_sb[:])
                    rsq = fsb.tile([128, 2, 512], BF16, tag="rsq", bufs=1,
                                   name="rsq")
                    with nc.allow_low_precision(reason="rms scale in bf16"):
                        nc.vector.reciprocal(rsq, rsq32)
                    nc.vector.tensor_tensor(
                        qn[:, 0, sl], rpq[:, 0, :], rsq[:, 0, :], AL.mult
                    )
                    nc.gpsimd.tensor_tensor(
                        qn[:, 1, sl], rpq[:, 1, :], rsq[:, 1, :], AL.mult
                    )

                if k_first:
                    k_side()
                    q_side()
                else:
                    q_side()
                    k_side()

            # ---- front phase: so-chunks 0..3, rms at 2-so lag ---------
            with (
                tc.tile_pool(name="fsb", bufs=1) as fsb,
                tc.tile_pool(name="fps", bufs=1, space="PSUM") as fp,
            ):
                # PE p-state pre-warm: ~3us of dummy matmuls during the DMA
                # lead-in ramp the tensor engine to full clock (the cost
                # model ramps 0.65 -> 1.2 -> 2.4GHz over 3us of continuous
                # busy and does not reset), so real matmuls run at 2.4GHz
                pwm = fp.tile([128, 2, 512], F32, tag="psq", bufs=1,
                              name="pwm")
                for w in range(26):
                    nc.tensor.matmul(
                        pwm[:, 0, 0:128], lhsT=onesq[:], rhs=onesq[:],
                        start=(w == 0), stop=(w == 25),
                    )

                def emit_proj(so, psr=None):
                    sl = slice(so * 512, (so + 1) * 512)
                    psq = fp.tile([128, 2, 512], F32, tag="psq", bufs=1,
                                  name="psq")
                    for ko in range(8):
                        for ch in range(2):
                            nc.tensor.matmul(
                                psq[:, ch, :],
                                lhsT=wq_sb[:, ko, ch * 128:(ch + 1) * 128],
                                rhs=hT_sb[:, ko, sl],
                                start=(ko == 0),
                                stop=(ko == 7) if not use_bias else False,
                            )
                    if use_bias:
                        for ch in range(2):
                            nc.tensor.matmul(
                                psq[:, ch, :],
                                lhsT=bias_sb[:, ch * 128:(ch + 1) * 128],
                                rhs=ones_row[:],
                                start=False, stop=True,
                            )
                    if psr is not None:
                        emit_psr_front(psr)
                    psk = fp.tile([128, 512], F32, tag="psk", bufs=1,
                                  name="psk")
                    for ko in range(8):
                        nc.tensor.matmul(
                            psk[:],
                            lhsT=wkv_sb[:, ko, 0:128],
                            rhs=hT_sb[:, ko, sl],
                            start=(ko == 0),
                            stop=(ko == 7) if not use_bias else False,
                        )
                    if use_bias:
                        nc.tensor.matmul(
                            psk[:],
                            lhsT=bias_sb[:, 256:384],
                            rhs=ones_row[:],
                            start=False, stop=True,
                        )
                    return psq, psk

                def emit_psr_front(so):
                    psrq = fp.tile([128, 2, 512], F32, tag="psrq", bufs=1,
                                   name="psrq")
                    psrk = fp.tile([128, 512], F32, tag="psrk", bufs=1,
                                   name="psrk")
                    emit_rms(so, psrq, psrk, fsb)

                # psr(so-2) sits between psq(so) and psk(so): the extra
                # psq time covers the rope/square chain latency of so-2
                for so in range(4):
                    psq, psk = emit_proj(so, psr=(so - 2) if so >= 2 else None)
                    emit_rope(so, psq, psk, fsb)
                emit_psr_front(2)
                # all four V groups ride in the front pool as PE filler
                # over the so2/so3 rms chains (the attention pool's first
                # allocs then hide behind the psr3 chain)
                for g in range(4):
                    psv = fp.tile([128, 4, 128], F32, tag="psv", bufs=2,
                                  name="psv")
                    psv_group(g, psv, one_bank=True)

            # ---- attention + psv/psr(3) filler + interleaved out_proj --
            with (
                tc.tile_pool(name="asb", bufs=1) as ab,
                tc.tile_pool(name="aps", bufs=1, space="PSUM") as ap,
            ):
                pso: dict = {}
                pT: dict = {}

                def emit_psr_attn(so):
                    psrq = ap.tile([128, 4, ICS], F32, tag="pss", bufs=3,
                                   name="psrq3").rearrange(
                        "p a b -> p (a b)"
                    ).rearrange("p (c d) -> p c d", c=2)
                    psrk = ap.tile([128, 2, 4, 64], F32, tag="pso", bufs=1,
                                   name="psrk3").rearrange(
                        "p a b c -> p (a b c)"
                    )
                    emit_rms(so, psrq, psrk, ab)

                # (half, ch) -> kn packing with that kv head at the right base
                def knt_for(half, ch):
                    return kn if (ch == 0) == (half == 0) else kn_sw

                def scores_exp(g):
                    ic, jc = divmod(g, NJC)
                    isl = slice(ic * ICS, (ic + 1) * ICS)
                    pss = ap.tile([128, 4, ICS], F32, tag="pss", bufs=3,
                                  name="pss")
                    for hs in range(4):
                        half, ch = divmod(hs, 2)
                        rows = slice(half * 64, half * 64 + 64)
                        nc.tensor.matmul(
                            pss[:, hs, :],
                            lhsT=knt_for(half, ch)[rows,
                                                   jc * 128:(jc + 1) * 128],
                            rhs=qn[rows, ch, isl],
                            start=(ch == 0), stop=(ch == 1),
                        )
                    pt = ab.tile([128, 4, ICS], BF16, tag="pT", bufs=LK + 2,
                                 name="pt")
                    if use_mask:
                        mkt = ab.tile([128, ICS], F32, tag="mkt", bufs=2,
                                      name="mkt")
                        nc.sync.dma_start(out=mkt[:], in_=mk[jc][:, isl])
                        sm = ab.tile([128, 4, ICS], F32, tag="sm", bufs=2,
                                     name="sm")
                        nc.vector.scalar_tensor_tensor(
                            sm, pss[:], 0.125,
                            mkt[:, None, :].to_broadcast((128, 4, ICS)),
                            AL.mult, AL.add,
                        )
                        nc.scalar.activation(pt, sm, AF.Exp)
                    elif JC_ENG[jc] == "act":
                        nc.scalar.activation(pt, pss, AF.Exp, scale=0.125)
                    else:
                        nc.vector.tensor_scalar(
                            pt.bitcast(I16), pss[:], SCH_A, SCH_B,
                            AL.mult, AL.add,
                        )
                    pT[g] = pt

                def emit_finish(ic):
                    """reciprocal + per-isub normalize + XBAR transposes
                    (per-isub so the first transposes fire early)."""
                    po, de = pso.pop(ic)
                    rcp = ab.tile([128, 2, 4, 1], F32, tag="rcp", bufs=2,
                                  name="rcp")
                    nc.vector.reciprocal(rcp[:], de[:, :, :, 0:1])
                    on = ab.tile([128, 2, 4, 64], BF16, tag="onat", bufs=2,
                                 name="on")
                    for isub in range(2):
                        nc.vector.tensor_tensor(
                            on[:, isub], po[:, isub],
                            rcp[:, isub].to_broadcast((128, 4, 64)),
                            AL.mult,
                        )
                        for cc in range(2):
                            nc.sync.dma_start_transpose(
                                oT[:, cc, ic,
                                   isub * 128:(isub + 1) * 128],
                                on[:, isub,
                                   2 * cc:2 * cc + 2, :].rearrange(
                                    "p a b -> p (a b)"
                                ),
                            )

                def emit_tail(ic):
                    """drain: PE transposes via identity (no XBAR DMA
                    latency), out_proj pieces interleaved per isub, piece
                    copies on ACT so the DVE chain stays short."""
                    po, de = pso.pop(ic)
                    rcp = ab.tile([128, 2, 4, 1], F32, tag="rcp", bufs=2,
                                  name="rcp")
                    nc.vector.reciprocal(rcp[:], de[:, :, :, 0:1])
                    on = ab.tile([128, 2, 4, 64], BF16, tag="onat", bufs=2,
                                 name="on")
                    psT = ap.tile([128, 2, 4, 64], F32, tag="den", bufs=1,
                                  name="psT")
                    psTf = psT.rearrange("p a b c -> p (a b c)")
                    for isub in range(2):
                        nc.vector.tensor_tensor(
                            on[:, isub], po[:, isub],
                            rcp[:, isub].to_broadcast((128, 4, 64)),
                            AL.mult,
                        )
                        for cc in range(2):
                            k = isub * 2 + cc
                            nc.tensor.matmul(
                                psTf[:, k * 128:(k + 1) * 128],
                                lhsT=on[:, isub,
                                        2 * cc:2 * cc + 2, :].rearrange(
                                    "p a b -> p (a b)"),
                                rhs=ident_sb[:],
                                start=(k == 0), stop=(k == 3),
                            )
                        nc.vector.tensor_copy(
                            oT[:, :, ic, isub * 128:(isub + 1) * 128],
                            psT[:, isub, :, :].rearrange(
                                "p c d -> p (c d)"
                            ).rearrange("p (c d) -> p c d", c=2),
                        )
                        for ec in range(2):
                            outproj_piece(ic, isub * 2 + ec, tail=True)

                def pv(r):
                    ric, rjc = divmod(r, NJC)
                    po, de = pso[ric]
                    pt = pT.pop(r)
                    for isub in range(2):
                        for hs in range(4):
                            first = rjc == 0 and isub == 0 and hs == 0
                            last = rjc == NJC - 1 and isub == 1 and hs == 3
                            nc.tensor.matmul(
                                po[:, isub, hs, :],
                                lhsT=pt[:, hs, isub * 128:(isub + 1) * 128],
                                rhs=v_all[:, rjc, hs % 2, 0:64],
                                start=first, stop=last,
                            )
                            nc.tensor.matmul(
                                de[:, isub, hs, 0:1],
                                lhsT=pt[:, hs, isub * 128:(isub + 1) * 128],
                                rhs=v_all[:, rjc, hs % 2, 64:65],
                                start=first, stop=last,
                            )

                def outproj_piece(ic, k, tail=False):
                    isub, ec = divmod(k, 2)
                    psy = ap.tile([128, 4, ICS], F32, tag="pss", bufs=3,
                                  name="psy")
                    out = psy[:, 0:2, :].rearrange("p a b -> p (a b)")
                    for cc in range(2):
                        nc.tensor.matmul(
                            out,
                            lhsT=oT[:, cc, ic, isub * 128:(isub + 1) * 128],
                            rhs=wo_sb[:, cc, ec * 512:(ec + 1) * 512],
                            start=(cc == 0), stop=(cc == 1),
                        )
                    dst = y_sb[:, ic, isub, ec * 512:(ec + 1) * 512]
                    if tail:
                        # drain: copies on ACT, per-ec DMAs so the final
                        # exposed chain is one copy + a small DMA
                        nc.scalar.copy(dst, out)
                        nc.sync.dma_start(
                            out=y[:, ic, isub, ec * 512:(ec + 1) * 512],
                            in_=dst,
                        )
                    else:
                        nc.vector.tensor_copy(dst, out)
                        if ec == 1:
                            nc.sync.dma_start(
                                out=y[:, ic, isub, :], in_=y_sb[:, ic, isub, :]
                            )

                # preamble: V groups 2/3 + the last rms chain ride in the
                # pss/pso rotations, absorbing the front-psum handoff;
                # exp table warm goes after the last front Sqrt so the ACT
                # function set switches exactly once
                emit_psr_attn(3)
                warm = cp.tile([1, 1], F32, name="warm")
                nc.scalar.activation(warm[:], eps_sb[0:1, :], AF.Exp)

                PIECE_JC = {8: 0, 10: 1, 12: 2, 15: 3}
                NG = NIC * NJC
                for g in range(NG + LK):
                    r = g - LK
                    if r >= 0 and r % NJC == 0:
                        ric = r // NJC
                        if ric >= 1:
                            emit_finish(ric - 1)
                        pso[ric] = (
                            ap.tile([128, 2, 4, 64], F32, tag="pso",
                                    bufs=1, name="pso"),
                            ap.tile([128, 2, 4, 64], F32, tag="den",
                                    bufs=1, name="den"),
                        )
                    if g < NG:
                        scores_exp(g)
                        ic, jc = divmod(g, NJC)
                        if ic >= 1 and jc in PIECE_JC:
                            outproj_piece(ic - 1, PIECE_JC[jc])
                    if r >= 0:
                        pv(r)
                emit_tail(NIC - 1)

    nc.compile()
    return nc


def _get(use_mask: bool, use_bias: bool):
    key = (use_mask, use_bias)
    if key not in _cache:
        _cache[key] = _build(use_mask, use_bias)
    return _cache[key]


def _host_prep(hidden_state, attention_mask, Wq, bq, Wk, bk, Wv, bv, Wo,
               use_mask, use_bias):
    half_q, half_k = HID // 2, (HKV * D) // 2  # 512, 128
    inv_q = ROPE_BASE ** (-np.arange(half_q, dtype=np.float64) / half_q)
    inv_k = ROPE_BASE ** (-np.arange(half_k, dtype=np.float64) / half_k)
    s_idx = np.arange(S, dtype=np.float64)
    cos_q = np.cos(inv_q[:, None] * s_idx[None, :])  # [512, S]
    sin_q = np.sin(inv_q[:, None] * s_idx[None, :])
    cos_k = np.cos(inv_k[:, None] * s_idx[None, :])  # [128, S]
    sin_k = np.sin(inv_k[:, None] * s_idx[None, :])

    in_maps = []
    for core in range(8):
        b, s = core // 4, core % 4
        qA = np.arange(128 * s, 128 * s + 128)       # chA q cols
        qB = qA + 512                                 # chB q cols
        kva = s // 2
        kA = np.arange(64 * kva, 64 * kva + 64)       # kv_a cols
        kB = kA + 128                                 # kv_b cols

        # hT layout: [hidden-dim-within-chunk, ko-chunk, S]
        hTc = np.ascontiguousarray(
            hidden_state[b].T.reshape(8, 128, S).transpose(1, 0, 2)
        ).astype(NB)
        wq_c = np.stack(
            [Wq[:, np.concatenate([qA, qB])][ko * 128:(ko + 1) * 128]
             for ko in range(8)], axis=1,
        ).astype(NB)  # [128, 8, 256]
        wk_cols = np.concatenate([Wk[:, kA], Wk[:, kB]], axis=1)  # [HID, 128]
        wv_cols = np.concatenate([Wv[:, kA], Wv[:, kB]], axis=1)  # [HID, 128]
        wkv_c = np.stack(
            [np.concatenate([wk_cols, wv_cols], axis=1)[ko * 128:(ko + 1) * 128]
             for ko in range(8)], axis=1,
        ).astype(NB)  # [128, 8, 256]
        worows = np.concatenate([qA[0:64], qB[0:64], qA[64:128], qB[64:128]])
        wo_c = Wo[worows].astype(NB).reshape(2, 128, HID).transpose(1, 0, 2)
        wo_c = np.ascontiguousarray(wo_c)
        qtab_c = np.stack(
            [cos_q[qA % 512], -sin_q[qA % 512], sin_q[qA % 512]], axis=1
        ).astype(NB)  # [128, 3, S]: cos, -sin, +sin
        # packed-K rope tables: cos duplicated to both partition halves;
        # sin with crossed signs (+sin on 0:64 feeds the upper-half output,
        # -sin on 64:128 feeds the lower-half output)
        kfreq = kA % 128
        ktab_c = np.stack(
            [np.concatenate([cos_k[kfreq], cos_k[kfreq]], axis=0),
             np.concatenate([sin_k[kfreq], -sin_k[kfreq]], axis=0)], axis=1
        ).astype(NB)  # [128, 2, S]

        m = {
            "hT": hTc, "wq": wq_c, "wkv": wkv_c, "wo": wo_c,
            "qtab": qtab_c, "ktab": ktab_c,
            "ident": np.eye(128, dtype=np.float32).astype(NB),
        }
        if use_bias:
            m["bias"] = np.concatenate(
                [bq[qA], bq[qB], bk[kA], bk[kB], bv[kA], bv[kB]]
            ).astype(np.float32).reshape(1, 512)
        if use_mask:
            mT = np.ascontiguousarray(attention_mask[b].T).astype(np.float32)
            m["mk"] = mT.reshape(NJC, 128, S)
        in_maps.append(m)
    return in_maps


def kernel(hidden_state, attention_mask, Wq, bq, Wk, bk, Wv, bv, Wo, bo):
    hidden_state = np.asarray(hidden_state, dtype=np.float32)
    attention_mask = np.asarray(attention_mask, dtype=np.float32)
    bq, bk, bv = (np.asarray(x, np.float32) for x in (bq, bk, bv))
    use_mask = bool(np.any(attention_mask))
    use_bias = bool(np.any(bq) or np.any(bk) or np.any(bv))
    nc = _get(use_mask, use_bias)
    in_maps = _host_prep(
        hidden_state, attention_mask,
        np.asarray(Wq, np.float32), bq,
        np.asarray(Wk, np.float32), bk,
        np.asarray(Wv, np.float32), bv,
        np.asarray(Wo, np.float32), use_mask, use_bias,
    )
    res = run_bass_kernel_spmd(nc, in_maps, list(range(8)))
    out = np.zeros((B, S, HID), dtype=np.float32)
    for core in range(8):
        yc = res.results[core]["y"].astype(np.float32)  # [128, NIC, 2, HID]
        out[core // 4] += yc.transpose(1, 2, 0, 3).reshape(S, HID)
    out += np.asarray(bo, np.float32)[None, None, :]
    return out


# revision 79
# speedup vs baseline: 1.0015x; 1.0015x over previous
"""GQA attention kernel for Trainium2, 8 NeuronCores.

Sharding: data-parallel over batch (B=2) x 4 head-shards -> 8 cores.
Shard s owns q-heads {2s, 2s+1, 2s+8, 2s+9}; heads h and h+8 are
rotate-half RoPE partners, so each shard's RoPE is self-contained.
Those 4 heads use kv-heads {s//2, s//2+2}, which are RoPE partners on
the K side.  out_proj is row-parallel; partials are summed on host.

v3 layout (cost-model driven):
  front: per-(so,ko) input DMAs ordered so PE starts ~2us in; K
    projection packed to 128 output partitions; rope via same-base
    DVE ops with output partition shift; rmsnorm via block-ones
    matmul + ACT Sqrt + DVE bf16 reciprocal; squares on Pool;
    kn_swapped built by base-shifted normalize ops (DVE/Pool).
  transition: V projections and the last rms chain ride in the
    attention PSUM pool's tag rotation as PE filler while the first
    scores steps issue - no dead zone between phases.
  attention: flat pipeline over 128 (ic, jc) steps; exp split
    ACT(10)/DVE-Schraudolph(6) per 16 jc (gpsimd cannot read PSUM);
    pss triple-buffered so scores(g+3) never waits on exp(g); PV with
    ones-column denominator; single fused normalize per ic; out_proj
    matmuls for ic-1 interleaved at jc 6/9/12/15 (psy lives in the
    pss tag rotation); oT via XBAR dma transpose; y copies on DVE.
"""

import numpy as np
import ml_dtypes

import concourse.bacc as bacc
import concourse.mybir as mybir
from concourse.tile import TileContext
from concourse.bass_utils import run_bass_kernel_spmd

BF16 = mybir.dt.bfloat16
F32 = mybir.dt.float32
I16 = mybir.dt.int16
AL = mybir.AluOpType
AF = mybir.ActivationFunctionType

# Schraudolph fast-exp in bf16 bit-space: exp(s/8) ~= bitcast_bf16(
# int16(SCH_A*s + SCH_B)).  RMS rel err ~1.8% on N(0,1) scores.
SCH_A = (2.0 ** 7 / np.log(2.0)) * 0.125
SCH_B = 127.0 * 2.0 ** 7 - 7.375

B, S, HID = 2, 2048, 1024
H, HKV, D = 16, 4, 64
ROPE_BASE = 10000.0
EPS = float(np.finfo(np.float32).eps)
NB = ml_dtypes.bfloat16

NIC = 8          # i-chunks of 256
ICS = S // NIC   # 256
NJC = 16         # j-chunks of 128
LK = 3           # PV lookahead in jc-steps

# per-jc exp engine within each i-chunk: ACT (exact) for most, DVE
# Schraudolph for the rest (Pool/gpsimd cannot read PSUM on TRN2).
# DVE steps sit away from the ic boundary (jc 14-15, 0) so the finish
# chain (reciprocal+normalize, also DVE) is not queued behind an exp.
JC_ENG = ["act"] * NJC
for _j in (3, 5, 7, 9, 11, 13):
    JC_ENG[_j] = "dve"

_cache: dict = {}


def _build(use_mask: bool, use_bias: bool):
    nc = bacc.Bacc("TRN2", target_bir_lowering=False)

    hT = nc.dram_tensor("hT", [128, 8, S], BF16, kind="ExternalInput")
    wq = nc.dram_tensor("wq", [128, 8, 256], BF16, kind="ExternalInput")
    wkv = nc.dram_tensor("wkv", [128, 8, 256], BF16, kind="ExternalInput")
    wo = nc.dram_tensor("wo", [128, 2, HID], BF16, kind="ExternalInput")
    # qtab: [cos, -sin, +sin]; ktab: [cos-dup, crossed-sign sin]
    qtab = nc.dram_tensor("qtab", [128, 3, S], BF16, kind="ExternalInput")
    ktab = nc.dram_tensor("ktab", [128, 2, S], BF16, kind="ExternalInput")
    ident = nc.dram_tensor("ident", [128, 128], BF16, kind="ExternalInput")
    if use_bias:
        bias = nc.dram_tensor("bias", [1, 512], F32, kind="ExternalInput")
    mk = (
        nc.dram_tensor("mk", [NJC, 128, S], F32, kind="ExternalInput")
        if use_mask
        else None
    )
    y = nc.dram_tensor("y", [128, NIC, 2, HID], BF16, kind="ExternalOutput")

    with TileContext(nc) as tc:
        with tc.tile_pool(name="const", bufs=1) as cp:
            # ---- persistent SBUF tiles --------------------------------
            hT_sb = cp.tile([128, 8, S], BF16)
            wq_sb = cp.tile([128, 8, 256], BF16)
            wkv_sb = cp.tile([128, 8, 256], BF16)  # [:, :, 0:128]=k, 128:256=v
            wo_sb = cp.tile([128, 2, HID], BF16)
            qco_sb = cp.tile([128, S], BF16)
            qsi2_sb = cp.tile([128, 2, S], BF16)  # [-sin, +sin]
            kco_sb = cp.tile([128, S], BF16)   # cos, duplicated halves
            ksi_sb = cp.tile([128, S], BF16)   # sin, crossed signs

            qn = cp.tile([128, 2, S], BF16)    # rmsnorm'd roped q
            kn = cp.tile([128, S], BF16)       # packed k: [kva d | kvb d]
            kn_sw = cp.tile([128, S], BF16)    # half-swapped copy
            v_all = cp.tile([128, NJC, 2, 65], BF16)  # v natural + ones col
            y_sb = cp.tile([128, NIC, 2, HID], BF16)
            oT = cp.tile([128, 2, NIC, ICS], BF16)  # [d-part, cc, ic, i]

            ident_sb = cp.tile([128, 128], BF16)
            eps_sb = cp.tile([128, 1], F32)
            nc.vector.memset(eps_sb[:], EPS)
            onesq = cp.tile([128, 128], BF16)  # block-diag 1/64
            nc.vector.memset(onesq[:], 0.0)
            nc.vector.memset(onesq[0:64, 0:64], 1.0 / 64.0)
            nc.vector.memset(onesq[64:128, 64:128], 1.0 / 64.0)
            nc.vector.memset(v_all[:], 1.0)
            if use_bias:
                ones_row = cp.tile([1, 512], BF16)
                nc.vector.memset(ones_row[:], 1.0)
                bias_sb = cp.tile([1, 512], F32)

            # ---- input DMAs: fine-grained, ordered for early PE start --
            # hT arrives per (so, ko); the previous quarter's rope tables
            # interleave one-per-hT-chunk so neither stream starves; wo
            # (needed only at out_proj) goes last.
            # HWDGE holds ~625ns per DMA, so few/large DMAs: hT per
            # (ko, S-half), rope tables per S-half, wq in two pieces.
            def tab_dmas(h):
                sl = slice(h * 1024, (h + 1) * 1024)
                nc.sync.dma_start(out=qco_sb[:, sl], in_=qtab[:, 0, sl])
                nc.sync.dma_start(out=qsi2_sb[:, :, sl], in_=qtab[:, 1:3, sl])
                nc.sync.dma_start(out=kco_sb[:, sl], in_=ktab[:, 0, sl])
                nc.sync.dma_start(out=ksi_sb[:, sl], in_=ktab[:, 1, sl])

            nc.sync.dma_start(out=wq_sb[:, 0, :], in_=wq[:, 0, :])
            nc.sync.dma_start(out=hT_sb[:, 0, 0:1024], in_=hT[:, 0, 0:1024])
            nc.sync.dma_start(out=wq_sb[:, 1:8, :], in_=wq[:, 1:8, :])
            nc.sync.dma_start(out=hT_sb[:, 1, 0:1024], in_=hT[:, 1, 0:1024])
            nc.sync.dma_start(out=wkv_sb[:], in_=wkv[:])
            if use_bias:
                nc.sync.dma_start(out=bias_sb[:], in_=bias[:])
            for ko in range(2, 8):
                nc.sync.dma_start(
                    out=hT_sb[:, ko, 0:1024], in_=hT[:, ko, 0:1024]
                )
            tab_dmas(0)
            for ko in range(8):
                nc.sync.dma_start(
                    out=hT_sb[:, ko, 1024:2048], in_=hT[:, ko, 1024:2048]
                )
            tab_dmas(1)
            nc.sync.dma_start(out=wo_sb[:], in_=wo[:])
            nc.sync.dma_start(out=ident_sb[:], in_=ident[:])

            chains = {}

            def psv_group(group, psv, one_bank):
                """V projection for 4 position-chunks into psv slices
                [:, i, 0:128]; accumulation-group flags per psum bank."""
                scs = range(4 * group, 4 * group + 4)
                for ko in range(8):
                    for i in range(4):
                        first = i == 0 or (not one_bank and i == 2)
                        last = i == 3 or (not one_bank and i == 1)
                        nc.tensor.matmul(
                            psv[:, i, 0:128],
                            lhsT=hT_sb[:, ko,
                                       scs[i] * 128:(scs[i] + 1) * 128],
                            rhs=wkv_sb[:, ko, 128:256],
                            start=(ko == 0 and first),
                            stop=(ko == 7 and last)
                            if not use_bias else False,
                        )
                if use_bias:
                    for i in range(4):
                        last = i == 3 or (not one_bank and i == 1)
                        nc.tensor.matmul(
                            psv[:, i, 0:128],
                            lhsT=ones_row[:, 0:128],
                            rhs=bias_sb[:, 384:512],
                            start=False, stop=last,
                        )
                for i in range(4):
                    nc.scalar.copy(
                        v_all[:, scs[i], :, 0:64],
                        psv[:, i, 0:128].rearrange("p (c d) -> p c d", c=2),
                    )

            def emit_rope(so, psq, psk, fsb):
                """ACT casts to bf16, DVE rope combines (inputs share a
                partition base; outputs shift), Pool squares."""
                sl = slice(so * 512, (so + 1) * 512)
                rawq = fsb.tile([128, 2, 512], BF16, tag="rawq", bufs=1,
                                name="rawq")
                nc.scalar.copy(rawq, psq)
                rawk = fsb.tile([128, 512], BF16, tag="rawk", bufs=1,
                                name="rawk")
                nc.scalar.copy(rawk, psk)

                uq = fsb.tile([128, 2, 512], BF16, tag="uq", bufs=1,
                              name="uq")
                nc.vector.tensor_tensor(
                    uq, rawq,
                    qco_sb[:, None, sl].to_broadcast((128, 2, 512)),
                    AL.mult,
                )
                # signs are folded into qsi2 ([-sin, +sin]) so one add
                # combines both rope halves
                wq_t = fsb.tile([128, 2, 512], BF16, tag="wq_t", bufs=1,
                                name="wq_t")
                nc.vector.tensor_tensor(
                    wq_t[:, 0, :], rawq[:, 1, :], qsi2_sb[:, 0, sl], AL.mult
                )
                nc.vector.tensor_tensor(
                    wq_t[:, 1, :], rawq[:, 0, :], qsi2_sb[:, 1, sl], AL.mult
                )
                rpq = cp.tile([128, 2, 512], BF16, name=f"rpq{so}")
                nc.vector.tensor_tensor(rpq, uq, wq_t, AL.add)
                sqq = cp.tile([128, 2, 512], BF16, name=f"sqq{so}")
                # so3's squares on DVE: its chain gates the attention start
                sq_eng = nc.vector if so == 3 else nc.gpsimd
                sq_eng.tensor_tensor(sqq, rpq, rpq, AL.mult)

                tk = fsb.tile([128, 512], BF16, tag="tk", bufs=1, name="tk")
                nc.vector.tensor_tensor(tk, rawk, kco_sb[:, sl], AL.mult)
                # ksi carries crossed signs (+sin on parts 0:64, -sin on
                # 64:128): both inputs share a partition base (HW rule),
                # only the output is base-shifted, and one add combines.
                wk_t = fsb.tile([128, 512], BF16, tag="wk_t", bufs=1,
                                name="wk_t")
                nc.vector.tensor_tensor(
                    wk_t[0:64, :], rawk[64:128, :], ksi_sb[64:128, sl],
                    AL.mult,
                )
                nc.vector.tensor_tensor(
                    wk_t[64:128, :], rawk[0:64, :], ksi_sb[0:64, sl],
                    AL.mult,
                )
                rpk = cp.tile([128, 512], BF16, name=f"rpk{so}")
                nc.vector.tensor_tensor(rpk, tk, wk_t, AL.add)
                sqk = cp.tile([128, 512], BF16, name=f"sqk{so}")
                sq_eng.tensor_tensor(sqk, rpk, rpk, AL.mult)
                chains[so] = (rpq, sqq, rpk, sqk)

            def emit_rms(so, psrq, psrk, fsb):
                """Sqrt on ACT, bf16 reciprocal on DVE, normalize multiplies
                on DVE/Pool; K side first (the last chunk's kn gates the
                attention start); kn_sw written with shifted output bases."""
                sl = slice(so * 512, (so + 1) * 512)
                rpq, sqq, rpk, sqk = chains.pop(so)
                k_first = so == 3
                if k_first:
                    nc.tensor.matmul(
                        psrk[:], lhsT=onesq[:], rhs=sqk[:],
                        start=True, stop=True,
                    )
                for ch in range(2):
                    nc.tensor.matmul(
                        psrq[:, ch, :], lhsT=onesq[:], rhs=sqq[:, ch, :],
                        start=True, stop=True,
                    )
                if not k_first:
                    nc.tensor.matmul(
                        psrk[:], lhsT=onesq[:], rhs=sqk[:],
                        start=True, stop=True,
                    )

                def k_side():
                    rsk32 = fsb.tile([128, 512], F32, tag="rsk32", bufs=1,
                                     name="rsk32")
                    nc.scalar.activation(rsk32, psrk, AF.Sqrt, bias=eps_sb[:])
                    rsk = fsb.tile([128, 512], BF16, tag="rsk", bufs=1,
                                   name="rsk")
                    with nc.allow_low_precision(reason="rms scale in bf16"):
                        nc.vector.reciprocal(rsk, rsk32)
                    nc.vector.tensor_tensor(kn[:, sl], rpk, rsk, AL.mult)
                    nc.vector.tensor_tensor(
                        kn_sw[0:64, sl], rpk[64:128, :], rsk[64:128, :],
                        AL.mult,
                    )
                    nc.gpsimd.tensor_tensor(
                        kn_sw[64:128, sl], rpk[0:64, :], rsk[0:64, :],
                        AL.mult,
                    )

                def q_side():
                    rsq32 = fsb.tile([128, 2, 512], F32, tag="rsq32", bufs=1,
                                     name="rsq32")
                    nc.scalar.activation(rsq32, psrq, AF.Sqrt, bias=eps_sb[:])
                    rsq = fsb.tile([128, 2, 512], BF16, tag="rsq", bufs=1,
                                   name="rsq")
                    with nc.allow_low_precision(reason="rms scale in bf16"):
                        nc.vector.reciprocal(rsq, rsq32)
                    nc.vector.tensor_tensor(
                        qn[:, 0, sl], rpq[:, 0, :], rsq[:, 0, :], AL.mult
                    )
                    nc.gpsimd.tensor_tensor(
                        qn[:, 1, sl], rpq[:, 1, :], rsq[:, 1, :], AL.mult
                    )

                if k_first:
                    k_side()
                    q_side()
                else:
                    q_side()
                    k_side()

            # ---- front phase: so-chunks 0..3, rms at 2-so lag ---------
            with (
                tc.tile_pool(name="fsb", bufs=1) as fsb,
                tc.tile_pool(name="fps", bufs=1, space="PSUM") as fp,
            ):
                # PE p-state pre-warm: ~3us of dummy matmuls during the DMA
                # lead-in ramp the tensor engine to full clock (the cost
                # model ramps 0.65 -> 1.2 -> 2.4GHz over 3us of continuous
                # busy and does not reset), so real matmuls run at 2.4GHz
                pwm = fp.tile([128, 2, 512], F32, tag="psq", bufs=1,
                              name="pwm")
                for w in range(26):
                    nc.tensor.matmul(
                        pwm[:, 0, 0:128], lhsT=onesq[:], rhs=onesq[:],
                        start=(w == 0), stop=(w == 25),
                    )

                def emit_proj(so, psr=None):
                    sl = slice(so * 512, (so + 1) * 512)
                    psq = fp.tile([128, 2, 512], F32, tag="psq", bufs=1,
                                  name="psq")
                    for ko in range(8):
                        for ch in range(2):
                            nc.tensor.matmul(
                                psq[:, ch, :],
                                lhsT=wq_sb[:, ko, ch * 128:(ch + 1) * 128],
                                rhs=hT_sb[:, ko, sl],
                                start=(ko == 0),
                                stop=(ko == 7) if not use_bias else False,
                            )
                    if use_bias:
                        for ch in range(2):
                            nc.tensor.matmul(
                                psq[:, ch, :],
                                lhsT=bias_sb[:, ch * 128:(ch + 1) * 128],
                                rhs=ones_row[:],
                                start=False, stop=True,
                            )
                    if psr is not None:
                        emit_psr_front(psr)
                    psk = fp.tile([128, 512], F32, tag="psk", bufs=1,
                                  name="psk")
                    for ko in range(8):
                        nc.tensor.matmul(
                            psk[:],
                            lhsT=wkv_sb[:, ko, 0:128],
                            rhs=hT_sb[:, ko, sl],
                            start=(ko == 0),
                            stop=(ko == 7) if not use_bias else False,
                        )
                    if use_bias:
                        nc.tensor.matmul(
                            psk[:],
                            lhsT=bias_sb[:, 256:384],
                            rhs=ones_row[:],
                            start=False, stop=True,
                        )
                    return psq, psk

                def emit_psr_front(so):
                    psrq = fp.tile([128, 2, 512], F32, tag="psrq", bufs=1,
                                   name="psrq")
                    psrk = fp.tile([128, 512], F32, tag="psrk", bufs=1,
                                   name="psrk")
                    emit_rms(so, psrq, psrk, fsb)

                # psr(so-2) sits between psq(so) and psk(so): the extra
                # psq time covers the rope/square chain latency of so-2
                for so in range(4):
                    psq, psk = emit_proj(so, psr=(so - 2) if so >= 2 else None)
                    emit_rope(so, psq, psk, fsb)
                emit_psr_front(2)
                # first two V groups ride in the front pool as PE filler
                # over the so2/so3 rms chains
                for g in (0, 1):
                    psv = fp.tile([128, 4, 128], F32, tag="psv", bufs=2,
                                  name="psv")
                    psv_group(g, psv, one_bank=True)

            # ---- attention + psv/psr(3) filler + interleaved out_proj --
            with (
                tc.tile_pool(name="asb", bufs=1) as ab,
                tc.tile_pool(name="aps", bufs=1, space="PSUM") as ap,
            ):
                pso: dict = {}
                pT: dict = {}

                def emit_psr_attn(so):
                    psrq = ap.tile([128, 4, ICS], F32, tag="pss", bufs=3,
                                   name="psrq3").rearrange(
                        "p a b -> p (a b)"
                    ).rearrange("p (c d) -> p c d", c=2)
                    psrk = ap.tile([128, 2, 4, 64], F32, tag="pso", bufs=1,
                                   name="psrk3").rearrange(
                        "p a b c -> p (a b c)"
                    )
                    emit_rms(so, psrq, psrk, ab)

                # (half, ch) -> kn packing with that kv head at the right base
                def knt_for(half, ch):
                    return kn if (ch == 0) == (half == 0) else kn_sw

                def scores_exp(g):
                    ic, jc = divmod(g, NJC)
                    isl = slice(ic * ICS, (ic + 1) * ICS)
                    pss = ap.tile([128, 4, ICS], F32, tag="pss", bufs=3,
                                  name="pss")
                    for hs in range(4):
                        half, ch = divmod(hs, 2)
                        rows = slice(half * 64, half * 64 + 64)
                        nc.tensor.matmul(
                            pss[:, hs, :],
                            lhsT=knt_for(half, ch)[rows,
                                                   jc * 128:(jc + 1) * 128],
                            rhs=qn[rows, ch, isl],
                            start=(ch == 0), stop=(ch == 1),
                        )
                    pt = ab.tile([128, 4, ICS], BF16, tag="pT", bufs=LK + 2,
                                 name="pt")
                    if use_mask:
                        mkt = ab.tile([128, ICS], F32, tag="mkt", bufs=2,
                                      name="mkt")
                        nc.sync.dma_start(out=mkt[:], in_=mk[jc][:, isl])
                        sm = ab.tile([128, 4, ICS], F32, tag="sm", bufs=2,
                                     name="sm")
                        nc.vector.scalar_tensor_tensor(
                            sm, pss[:], 0.125,
                            mkt[:, None, :].to_broadcast((128, 4, ICS)),
                            AL.mult, AL.add,
                        )
                        nc.scalar.activation(pt, sm, AF.Exp)
                    elif JC_ENG[jc] == "act":
                        nc.scalar.activation(pt, pss, AF.Exp, scale=0.125)
                    else:
                        nc.vector.tensor_scalar(
                            pt.bitcast(I16), pss[:], SCH_A, SCH_B,
                            AL.mult, AL.add,
                        )
                    pT[g] = pt

                def emit_finish(ic):
                    """reciprocal + per-isub normalize + XBAR transposes
                    (per-isub so the first transposes fire early)."""
                    po, de = pso.pop(ic)
                    rcp = ab.tile([128, 2, 4, 1], F32, tag="rcp", bufs=2,
                                  name="rcp")
                    nc.vector.reciprocal(rcp[:], de[:, :, :, 0:1])
                    on = ab.tile([128, 2, 4, 64], BF16, tag="onat", bufs=2,
                                 name="on")
                    for isub in range(2):
                        nc.vector.tensor_tensor(
                            on[:, isub], po[:, isub],
                            rcp[:, isub].to_broadcast((128, 4, 64)),
                            AL.mult,
                        )
                        for cc in range(2):
                            nc.sync.dma_start_transpose(
                                oT[:, cc, ic,
                                   isub * 128:(isub + 1) * 128],
                                on[:, isub,
                                   2 * cc:2 * cc + 2, :].rearrange(
                                    "p a b -> p (a b)"
                                ),
                            )

                def emit_tail(ic):
                    """drain: PE transposes via identity (no XBAR DMA
                    latency), out_proj pieces interleaved per isub, piece
                    copies on ACT so the DVE chain stays short."""
                    po, de = pso.pop(ic)
                    rcp = ab.tile([128, 2, 4, 1], F32, tag="rcp", bufs=2,
                                  name="rcp")
                    nc.vector.reciprocal(rcp[:], de[:, :, :, 0:1])
                    on = ab.tile([128, 2, 4, 64], BF16, tag="onat", bufs=2,
                                 name="on")
                    psT = ap.tile([128, 2, 4, 64], F32, tag="den", bufs=1,
                                  name="psT")
                    psTf = psT.rearrange("p a b c -> p (a b c)")
                    for isub in range(2):
                        nc.vector.tensor_tensor(
                            on[:, isub], po[:, isub],
                            rcp[:, isub].to_broadcast((128, 4, 64)),
                            AL.mult,
                        )
                        for cc in range(2):
                            k = isub * 2 + cc
                            nc.tensor.matmul(
                                psTf[:, k * 128:(k + 1) * 128],
                                lhsT=on[:, isub,
                                        2 * cc:2 * cc + 2, :].rearrange(
                                    "p a b -> p (a b)"),
                                rhs=ident_sb[:],
                                start=(k == 0), stop=(k == 3),
                            )
                        nc.vector.tensor_copy(
                            oT[:, :, ic, isub * 128:(isub + 1) * 128],
                            psT[:, isub, :, :].rearrange(
                                "p c d -> p (c d)"
                            ).rearrange("p (c d) -> p c d", c=2),
                        )
                        for ec in range(2):
                            outproj_piece(ic, isub * 2 + ec, tail=True)

                def pv(r):
                    ric, rjc = divmod(r, NJC)
                    po, de = pso[ric]
                    pt = pT.pop(r)
                    for isub in range(2):
                        for hs in range(4):
                            first = rjc == 0 and isub == 0 and hs == 0
                            last = rjc == NJC - 1 and isub == 1 and hs == 3
                            nc.tensor.matmul(
                                po[:, isub, hs, :],
                                lhsT=pt[:, hs, isub * 128:(isub + 1) * 128],
                                rhs=v_all[:, rjc, hs % 2, 0:64],
                                start=first, stop=last,
                            )
                            nc.tensor.matmul(
                                de[:, isub, hs, 0:1],
                                lhsT=pt[:, hs, isub * 128:(isub + 1) * 128],
                                rhs=v_all[:, rjc, hs % 2, 64:65],
                                start=first, stop=last,
                            )

                def outproj_piece(ic, k, tail=False):
                    isub, ec = divmod(k, 2)
                    psy = ap.tile([128, 4, ICS], F32, tag="pss", bufs=3,
                                  name="psy")
                    out = psy[:, 0:2, :].rearrange("p a b -> p (a b)")
                    for cc in range(2):
                        nc.tensor.matmul(
                            out,
                            lhsT=oT[:, cc, ic, isub * 128:(isub + 1) * 128],
                            rhs=wo_sb[:, cc, ec * 512:(ec + 1) * 512],
                            start=(cc == 0), stop=(cc == 1),
                        )
                    dst = y_sb[:, ic, isub, ec * 512:(ec + 1) * 512]
                    if tail:
                        # drain: copies on ACT, per-ec DMAs so the final
                        # exposed chain is one copy + a small DMA
                        nc.scalar.copy(dst, out)
                        nc.sync.dma_start(
                            out=y[:, ic, isub, ec * 512:(ec + 1) * 512],
                            in_=dst,
                        )
                    else:
                        nc.vector.tensor_copy(dst, out)
                        if ec == 1:
                            nc.sync.dma_start(
                                out=y[:, ic, isub, :], in_=y_sb[:, ic, isub, :]
                            )

                # preamble: V groups 2/3 + the last rms chain ride in the
                # pss/pso rotations, absorbing the front-psum handoff;
                # exp table warm goes after the last front Sqrt so the ACT
                # function set switches exactly once
                for g23 in (2, 3):
                    psvt = ap.tile([128, 4, ICS], F32, tag="pss", bufs=3,
                                   name="psvt")
                    psv_group(g23, psvt, one_bank=False)
                emit_psr_attn(3)
                warm = cp.tile([1, 1], F32, name="warm")
                nc.scalar.activation(warm[:], eps_sb[0:1, :], AF.Exp)

                PIECE_JC = {8: 0, 10: 1, 12: 2, 15: 3}
                NG = NIC * NJC
                for g in range(NG + LK):
                    r = g - LK
                    if r >= 0 and r % NJC == 0:
                        ric = r // NJC
                        if ric >= 1:
                            emit_finish(ric - 1)
                        pso[ric] = (
                            ap.tile([128, 2, 4, 64], F32, tag="pso",
                                    bufs=1, name="pso"),
                            ap.tile([128, 2, 4, 64], F32, tag="den",
                                    bufs=1, name="den"),
                        )
                    if g < NG:
                        scores_exp(g)
                        ic, jc = divmod(g, NJC)
                        if ic >= 1 and jc in PIECE_JC:
                            outproj_piece(ic - 1, PIECE_JC[jc])
                    if r >= 0:
                        pv(r)
                emit_tail(NIC - 1)

    nc.compile()
    return nc


def _get(use_mask: bool, use_bias: bool):
    key = (use_mask, use_bias)
    if key not in _cache:
        _cache[key] = _build(use_mask, use_bias)
    return _cache[key]


def _host_prep(hidden_state, attention_mask, Wq, bq, Wk, bk, Wv, bv, Wo,
               use_mask, use_bias):
    half_q, half_k = HID // 2, (HKV * D) // 2  # 512, 128
    inv_q = ROPE_BASE ** (-np.arange(half_q, dtype=np.float64) / half_q)
    inv_k = ROPE_BASE ** (-np.arange(half_k, dtype=np.float64) / half_k)
    s_idx = np.arange(S, dtype=np.float64)
    cos_q = np.cos(inv_q[:, None] * s_idx[None, :])  # [512, S]
    sin_q = np.sin(inv_q[:, None] * s_idx[None, :])
    cos_k = np.cos(inv_k[:, None] * s_idx[None, :])  # [128, S]
    sin_k = np.sin(inv_k[:, None] * s_idx[None, :])

    in_maps = []
    for core in range(8):
        b, s = core // 4, core % 4
        qA = np.arange(128 * s, 128 * s + 128)       # chA q cols
        qB = qA + 512                                 # chB q cols
        kva = s // 2
        kA = np.arange(64 * kva, 64 * kva + 64)       # kv_a cols
        kB = kA + 128                                 # kv_b cols

        # hT layout: [hidden-dim-within-chunk, ko-chunk, S]
        hTc = np.ascontiguousarray(
            hidden_state[b].T.reshape(8, 128, S).transpose(1, 0, 2)
        ).astype(NB)
        wq_c = np.stack(
            [Wq[:, np.concatenate([qA, qB])][ko * 128:(ko + 1) * 128]
             for ko in range(8)], axis=1,
        ).astype(NB)  # [128, 8, 256]
        wk_cols = np.concatenate([Wk[:, kA], Wk[:, kB]], axis=1)  # [HID, 128]
        wv_cols = np.concatenate([Wv[:, kA], Wv[:, kB]], axis=1)  # [HID, 128]
        wkv_c = np.stack(
            [np.concatenate([wk_cols, wv_cols], axis=1)[ko * 128:(ko + 1) * 128]
             for ko in range(8)], axis=1,
        ).astype(NB)  # [128, 8, 256]
        worows = np.concatenate([qA[0:64], qB[0:64], qA[64:128], qB[64:128]])
        wo_c = Wo[worows].astype(NB).reshape(2, 128, HID).transpose(1, 0, 2)
        wo_c = np.ascontiguousarray(wo_c)
        qtab_c = np.stack(
            [cos_q[qA % 512], -sin_q[qA % 512], sin_q[qA % 512]], axis=1
        ).astype(NB)  # [128, 3, S]: cos, -sin, +sin
        # packed-K rope tables: cos duplicated to both partition halves;
        # sin with crossed signs (+sin on 0:64 feeds the upper-half output,
        # -sin on 64:128 feeds the lower-half output)
        kfreq = kA % 128
        ktab_c = np.stack(
            [np.concatenate([cos_k[kfreq], cos_k[kfreq]], axis=0),
             np.concatenate([sin_k[kfreq], -sin_k[kfreq]], axis=0)], axis=1
        ).astype(NB)  # [128, 2, S]

        m = {
            "hT": hTc, "wq": wq_c, "wkv": wkv_c, "wo": wo_c,
            "qtab": qtab_c, "ktab": ktab_c,
            "ident": np.eye(128, dtype=np.float32).astype(NB),
        }
        if use_bias:
            m["bias"] = np.concatenate(
                [bq[qA], bq[qB], bk[kA], bk[kB], bv[kA], bv[kB]]
            ).astype(np.float32).reshape(1, 512)
        if use_mask:
            mT = np.ascontiguousarray(attention_mask[b].T).astype(np.float32)
            m["mk"] = mT.reshape(NJC, 128, S)
        in_maps.append(m)
    return in_maps


def kernel(hidden_state, attention_mask, Wq, bq, Wk, bk, Wv, bv, Wo, bo):
    hidden_state = np.asarray(hidden_state, dtype=np.float32)
    attention_mask = np.asarray(attention_mask, dtype=np.float32)
    bq, bk, bv = (np.asarray(x, np.float32) for x in (bq, bk, bv))
    use_mask = bool(np.any(attention_mask))
    use_bias = bool(np.any(bq) or np.any(bk) or np.any(bv))
    nc = _get(use_mask, use_bias)
    in_maps = _host_prep(
        hidden_state, attention_mask,
        np.asarray(Wq, np.float32), bq,
        np.asarray(Wk, np.float32), bk,
        np.asarray(Wv, np.float32), bv,
        np.asarray(Wo, np.float32), use_mask, use_bias,
    )
    res = run_bass_kernel_spmd(nc, in_maps, list(range(8)))
    out = np.zeros((B, S, HID), dtype=np.float32)
    for core in range(8):
        yc = res.results[core]["y"].astype(np.float32)  # [128, NIC, 2, HID]
        out[core // 4] += yc.transpose(1, 2, 0, 3).reshape(S, HID)
    out += np.asarray(bo, np.float32)[None, None, :]
    return out


# revision 80
# speedup vs baseline: 1.0128x; 1.0113x over previous
"""GQA attention kernel for Trainium2, 8 NeuronCores.

Sharding: data-parallel over batch (B=2) x 4 head-shards -> 8 cores.
Shard s owns q-heads {2s, 2s+1, 2s+8, 2s+9}; heads h and h+8 are
rotate-half RoPE partners, so each shard's RoPE is self-contained.
Those 4 heads use kv-heads {s//2, s//2+2}, which are RoPE partners on
the K side.  out_proj is row-parallel; partials are summed on host.

v3 layout (cost-model driven):
  front: per-(so,ko) input DMAs ordered so PE starts ~2us in; K
    projection packed to 128 output partitions; rope via same-base
    DVE ops with output partition shift; rmsnorm via block-ones
    matmul + ACT Sqrt + DVE bf16 reciprocal; squares on Pool;
    kn_swapped built by base-shifted normalize ops (DVE/Pool).
  transition: V projections and the last rms chain ride in the
    attention PSUM pool's tag rotation as PE filler while the first
    scores steps issue - no dead zone between phases.
  attention: flat pipeline over 128 (ic, jc) steps; exp split
    ACT(10)/DVE-Schraudolph(6) per 16 jc (gpsimd cannot read PSUM);
    pss triple-buffered so scores(g+3) never waits on exp(g); PV with
    ones-column denominator; single fused normalize per ic; out_proj
    matmuls for ic-1 interleaved at jc 6/9/12/15 (psy lives in the
    pss tag rotation); oT via XBAR dma transpose; y copies on DVE.
"""

import numpy as np
import ml_dtypes

import concourse.bacc as bacc
import concourse.mybir as mybir
from concourse.tile import TileContext
from concourse.bass_utils import run_bass_kernel_spmd

BF16 = mybir.dt.bfloat16
F32 = mybir.dt.float32
I16 = mybir.dt.int16
AL = mybir.AluOpType
AF = mybir.ActivationFunctionType

# Schraudolph fast-exp in bf16 bit-space: exp(s/8) ~= bitcast_bf16(
# int16(SCH_A*s + SCH_B)).  RMS rel err ~1.8% on N(0,1) scores.
SCH_A = (2.0 ** 7 / np.log(2.0)) * 0.125
SCH_B = 127.0 * 2.0 ** 7 - 7.375

B, S, HID = 2, 2048, 1024
H, HKV, D = 16, 4, 64
ROPE_BASE = 10000.0
EPS = float(np.finfo(np.float32).eps)
NB = ml_dtypes.bfloat16

NIC = 8          # i-chunks of 256
ICS = S // NIC   # 256
NJC = 16         # j-chunks of 128
LK = 3           # PV lookahead in jc-steps

# per-jc exp engine within each i-chunk: ACT (exact) for most, DVE
# Schraudolph for the rest (Pool/gpsimd cannot read PSUM on TRN2).
# DVE steps sit away from the ic boundary (jc 14-15, 0) so the finish
# chain (reciprocal+normalize, also DVE) is not queued behind an exp.
JC_ENG = ["act"] * NJC
for _j in (3, 5, 7, 9, 11, 13):
    JC_ENG[_j] = "dve"

_cache: dict = {}


def _build(use_mask: bool, use_bias: bool):
    nc = bacc.Bacc("TRN2", target_bir_lowering=False)

    hT = nc.dram_tensor("hT", [128, 8, S], BF16, kind="ExternalInput")
    wq = nc.dram_tensor("wq", [128, 8, 256], BF16, kind="ExternalInput")
    wkv = nc.dram_tensor("wkv", [128, 8, 256], BF16, kind="ExternalInput")
    wo = nc.dram_tensor("wo", [128, 2, HID], BF16, kind="ExternalInput")
    # qtab: [cos, -sin, +sin]; ktab: [cos-dup, crossed-sign sin]
    qtab = nc.dram_tensor("qtab", [128, 3, S], BF16, kind="ExternalInput")
    ktab = nc.dram_tensor("ktab", [128, 2, S], BF16, kind="ExternalInput")
    ident = nc.dram_tensor("ident", [128, 128], BF16, kind="ExternalInput")
    if use_bias:
        bias = nc.dram_tensor("bias", [1, 512], F32, kind="ExternalInput")
    mk = (
        nc.dram_tensor("mk", [NJC, 128, S], F32, kind="ExternalInput")
        if use_mask
        else None
    )
    y = nc.dram_tensor("y", [128, NIC, 2, HID], BF16, kind="ExternalOutput")

    with TileContext(nc) as tc:
        with tc.tile_pool(name="const", bufs=1) as cp:
            # ---- persistent SBUF tiles --------------------------------
            hT_sb = cp.tile([128, 8, S], BF16)
            wq_sb = cp.tile([128, 8, 256], BF16)
            wkv_sb = cp.tile([128, 8, 256], BF16)  # [:, :, 0:128]=k, 128:256=v
            wo_sb = cp.tile([128, 2, HID], BF16)
            qco_sb = cp.tile([128, S], BF16)
            qsi2_sb = cp.tile([128, 2, S], BF16)  # [-sin, +sin]
            kco_sb = cp.tile([128, S], BF16)   # cos, duplicated halves
            ksi_sb = cp.tile([128, S], BF16)   # sin, crossed signs

            qn = cp.tile([128, 2, S], BF16)    # rmsnorm'd roped q
            kn = cp.tile([128, S], BF16)       # packed k: [kva d | kvb d]
            kn_sw = cp.tile([128, S], BF16)    # half-swapped copy
            v_all = cp.tile([128, NJC, 2, 65], BF16)  # v natural + ones col
            y_sb = cp.tile([128, NIC, 2, HID], BF16)
            oT = cp.tile([128, 2, NIC, ICS], BF16)  # [d-part, cc, ic, i]

            ident_sb = cp.tile([128, 128], BF16)
            eps_sb = cp.tile([128, 1], F32)
            nc.vector.memset(eps_sb[:], EPS)
            onesq = cp.tile([128, 128], BF16)  # block-diag 1/64
            nc.vector.memset(onesq[:], 0.0)
            nc.vector.memset(onesq[0:64, 0:64], 1.0 / 64.0)
            nc.vector.memset(onesq[64:128, 64:128], 1.0 / 64.0)
            nc.vector.memset(v_all[:], 1.0)
            if use_bias:
                ones_row = cp.tile([1, 512], BF16)
                nc.vector.memset(ones_row[:], 1.0)
                bias_sb = cp.tile([1, 512], F32)

            # ---- input DMAs: fine-grained, ordered for early PE start --
            # hT arrives per (so, ko); the previous quarter's rope tables
            # interleave one-per-hT-chunk so neither stream starves; wo
            # (needed only at out_proj) goes last.
            # HWDGE holds ~625ns per DMA, so few/large DMAs: hT per
            # (ko, S-half), rope tables per S-half, wq in two pieces.
            def tab_dmas(h):
                sl = slice(h * 1024, (h + 1) * 1024)
                nc.sync.dma_start(out=qco_sb[:, sl], in_=qtab[:, 0, sl])
                nc.sync.dma_start(out=qsi2_sb[:, :, sl], in_=qtab[:, 1:3, sl])
                nc.sync.dma_start(out=kco_sb[:, sl], in_=ktab[:, 0, sl])
                nc.sync.dma_start(out=ksi_sb[:, sl], in_=ktab[:, 1, sl])

            nc.sync.dma_start(out=wq_sb[:, 0, :], in_=wq[:, 0, :])
            nc.sync.dma_start(out=hT_sb[:, 0, 0:1024], in_=hT[:, 0, 0:1024])
            nc.sync.dma_start(out=wq_sb[:, 1:8, :], in_=wq[:, 1:8, :])
            nc.sync.dma_start(out=hT_sb[:, 1, 0:1024], in_=hT[:, 1, 0:1024])
            nc.sync.dma_start(out=wkv_sb[:], in_=wkv[:])
            if use_bias:
                nc.sync.dma_start(out=bias_sb[:], in_=bias[:])
            for ko in range(2, 8):
                nc.sync.dma_start(
                    out=hT_sb[:, ko, 0:1024], in_=hT[:, ko, 0:1024]
                )
            tab_dmas(0)
            for ko in range(8):
                nc.sync.dma_start(
                    out=hT_sb[:, ko, 1024:2048], in_=hT[:, ko, 1024:2048]
                )
            tab_dmas(1)
            nc.sync.dma_start(out=wo_sb[:], in_=wo[:])
            nc.sync.dma_start(out=ident_sb[:], in_=ident[:])

            chains = {}

            def psv_group(group, psv, one_bank):
                """V projection for 4 position-chunks into psv slices
                [:, i, 0:128]; accumulation-group flags per psum bank."""
                scs = range(4 * group, 4 * group + 4)
                for ko in range(8):
                    for i in range(4):
                        first = i == 0 or (not one_bank and i == 2)
                        last = i == 3 or (not one_bank and i == 1)
                        nc.tensor.matmul(
                            psv[:, i, 0:128],
                            lhsT=hT_sb[:, ko,
                                       scs[i] * 128:(scs[i] + 1) * 128],
                            rhs=wkv_sb[:, ko, 128:256],
                            start=(ko == 0 and first),
                            stop=(ko == 7 and last)
                            if not use_bias else False,
                        )
                if use_bias:
                    for i in range(4):
                        last = i == 3 or (not one_bank and i == 1)
                        nc.tensor.matmul(
                            psv[:, i, 0:128],
                            lhsT=ones_row[:, 0:128],
                            rhs=bias_sb[:, 384:512],
                            start=False, stop=last,
                        )
                for i in range(4):
                    nc.scalar.copy(
                        v_all[:, scs[i], :, 0:64],
                        psv[:, i, 0:128].rearrange("p (c d) -> p c d", c=2),
                    )

            def emit_rope(so, psq, psk, fsb):
                """ACT casts to bf16, DVE rope combines (inputs share a
                partition base; outputs shift), Pool squares."""
                sl = slice(so * 512, (so + 1) * 512)
                rawq = fsb.tile([128, 2, 512], BF16, tag="rawq", bufs=1,
                                name="rawq")
                nc.scalar.copy(rawq, psq)
                rawk = fsb.tile([128, 512], BF16, tag="rawk", bufs=1,
                                name="rawk")
                nc.scalar.copy(rawk, psk)

                uq = fsb.tile([128, 2, 512], BF16, tag="uq", bufs=1,
                              name="uq")
                nc.vector.tensor_tensor(
                    uq, rawq,
                    qco_sb[:, None, sl].to_broadcast((128, 2, 512)),
                    AL.mult,
                )
                # signs are folded into qsi2 ([-sin, +sin]) so one add
                # combines both rope halves
                wq_t = fsb.tile([128, 2, 512], BF16, tag="wq_t", bufs=1,
                                name="wq_t")
                nc.vector.tensor_tensor(
                    wq_t[:, 0, :], rawq[:, 1, :], qsi2_sb[:, 0, sl], AL.mult
                )
                nc.vector.tensor_tensor(
                    wq_t[:, 1, :], rawq[:, 0, :], qsi2_sb[:, 1, sl], AL.mult
                )
                rpq = cp.tile([128, 2, 512], BF16, name=f"rpq{so}")
                nc.vector.tensor_tensor(rpq, uq, wq_t, AL.add)
                sqq = cp.tile([128, 2, 512], BF16, name=f"sqq{so}")
                # so3's squares on DVE: its chain gates the attention start
                sq_eng = nc.vector if so == 3 else nc.gpsimd
                sq_eng.tensor_tensor(sqq, rpq, rpq, AL.mult)

                tk = fsb.tile([128, 512], BF16, tag="tk", bufs=1, name="tk")
                nc.vector.tensor_tensor(tk, rawk, kco_sb[:, sl], AL.mult)
                # ksi carries crossed signs (+sin on parts 0:64, -sin on
                # 64:128): both inputs share a partition base (HW rule),
                # only the output is base-shifted, and one add combines.
                wk_t = fsb.tile([128, 512], BF16, tag="wk_t", bufs=1,
                                name="wk_t")
                nc.vector.tensor_tensor(
                    wk_t[0:64, :], rawk[64:128, :], ksi_sb[64:128, sl],
                    AL.mult,
                )
                nc.vector.tensor_tensor(
                    wk_t[64:128, :], rawk[0:64, :], ksi_sb[0:64, sl],
                    AL.mult,
                )
                rpk = cp.tile([128, 512], BF16, name=f"rpk{so}")
                nc.vector.tensor_tensor(rpk, tk, wk_t, AL.add)
                sqk = cp.tile([128, 512], BF16, name=f"sqk{so}")
                sq_eng.tensor_tensor(sqk, rpk, rpk, AL.mult)
                chains[so] = (rpq, sqq, rpk, sqk)

            def emit_rms(so, psrq, psrk, fsb):
                """Sqrt on ACT, bf16 reciprocal on DVE, normalize multiplies
                on DVE/Pool; K side first (the last chunk's kn gates the
                attention start); kn_sw written with shifted output bases."""
                sl = slice(so * 512, (so + 1) * 512)
                rpq, sqq, rpk, sqk = chains.pop(so)
                k_first = so == 3
                if k_first:
                    nc.tensor.matmul(
                        psrk[:], lhsT=onesq[:], rhs=sqk[:],
                        start=True, stop=True,
                    )
                for ch in range(2):
                    nc.tensor.matmul(
                        psrq[:, ch, :], lhsT=onesq[:], rhs=sqq[:, ch, :],
                        start=True, stop=True,
                    )
                if not k_first:
                    nc.tensor.matmul(
                        psrk[:], lhsT=onesq[:], rhs=sqk[:],
                        start=True, stop=True,
                    )

                def k_side():
                    rsk32 = fsb.tile([128, 512], F32, tag="rsk32", bufs=1,
                                     name="rsk32")
                    nc.scalar.activation(rsk32, psrk, AF.Sqrt, bias=eps_sb[:])
                    rsk = fsb.tile([128, 512], BF16, tag="rsk", bufs=1,
                                   name="rsk")
                    with nc.allow_low_precision(reason="rms scale in bf16"):
                        nc.vector.reciprocal(rsk, rsk32)
                    nc.vector.tensor_tensor(kn[:, sl], rpk, rsk, AL.mult)
                    nc.vector.tensor_tensor(
                        kn_sw[0:64, sl], rpk[64:128, :], rsk[64:128, :],
                        AL.mult,
                    )
                    nc.gpsimd.tensor_tensor(
                        kn_sw[64:128, sl], rpk[0:64, :], rsk[0:64, :],
                        AL.mult,
                    )

                def q_side():
                    rsq32 = fsb.tile([128, 2, 512], F32, tag="rsq32", bufs=1,
                                     name="rsq32")
                    nc.scalar.activation(rsq32, psrq, AF.Sqrt, bias=eps_sb[:])
                    rsq = fsb.tile([128, 2, 512], BF16, tag="rsq", bufs=1,
                                   name="rsq")
                    with nc.allow_low_precision(reason="rms scale in bf16"):
                        nc.vector.reciprocal(rsq, rsq32)
                    nc.vector.tensor_tensor(
                        qn[:, 0, sl], rpq[:, 0, :], rsq[:, 0, :], AL.mult
                    )
                    nc.gpsimd.tensor_tensor(
                        qn[:, 1, sl], rpq[:, 1, :], rsq[:, 1, :], AL.mult
                    )

                if k_first:
                    k_side()
                    q_side()
                else:
                    q_side()
                    k_side()

            # ---- front phase: so-chunks 0..3, rms at 2-so lag ---------
            with (
                tc.tile_pool(name="fsb", bufs=1) as fsb,
                tc.tile_pool(name="fps", bufs=1, space="PSUM") as fp,
            ):
                def emit_proj(so, psr=None):
                    sl = slice(so * 512, (so + 1) * 512)
                    psq = fp.tile([128, 2, 512], F32, tag="psq", bufs=1,
                                  name="psq")
                    for ko in range(8):
                        for ch in range(2):
                            nc.tensor.matmul(
                                psq[:, ch, :],
                                lhsT=wq_sb[:, ko, ch * 128:(ch + 1) * 128],
                                rhs=hT_sb[:, ko, sl],
                                start=(ko == 0),
                                stop=(ko == 7) if not use_bias else False,
                            )
                    if use_bias:
                        for ch in range(2):
                            nc.tensor.matmul(
                                psq[:, ch, :],
                                lhsT=bias_sb[:, ch * 128:(ch + 1) * 128],
                                rhs=ones_row[:],
                                start=False, stop=True,
                            )
                    if psr is not None:
                        emit_psr_front(psr)
                    psk = fp.tile([128, 512], F32, tag="psk", bufs=1,
                                  name="psk")
                    for ko in range(8):
                        nc.tensor.matmul(
                            psk[:],
                            lhsT=wkv_sb[:, ko, 0:128],
                            rhs=hT_sb[:, ko, sl],
                            start=(ko == 0),
                            stop=(ko == 7) if not use_bias else False,
                        )
                    if use_bias:
                        nc.tensor.matmul(
                            psk[:],
                            lhsT=bias_sb[:, 256:384],
                            rhs=ones_row[:],
                            start=False, stop=True,
                        )
                    return psq, psk

                def emit_psr_front(so):
                    psrq = fp.tile([128, 2, 512], F32, tag="psrq", bufs=1,
                                   name="psrq")
                    psrk = fp.tile([128, 512], F32, tag="psrk", bufs=1,
                                   name="psrk")
                    emit_rms(so, psrq, psrk, fsb)

                # psr(so-2) sits between psq(so) and psk(so): the extra
                # psq time covers the rope/square chain latency of so-2
                for so in range(4):
                    psq, psk = emit_proj(so, psr=(so - 2) if so >= 2 else None)
                    emit_rope(so, psq, psk, fsb)
                emit_psr_front(2)
                # first two V groups ride in the front pool as PE filler
                # over the so2/so3 rms chains
                for g in (0, 1):
                    psv = fp.tile([128, 4, 128], F32, tag="psv", bufs=2,
                                  name="psv")
                    psv_group(g, psv, one_bank=True)

            # ---- attention + psv/psr(3) filler + interleaved out_proj --
            with (
                tc.tile_pool(name="asb", bufs=1) as ab,
                tc.tile_pool(name="aps", bufs=1, space="PSUM") as ap,
            ):
                pso: dict = {}
                pT: dict = {}

                def emit_psr_attn(so):
                    psrq = ap.tile([128, 4, ICS], F32, tag="pss", bufs=3,
                                   name="psrq3").rearrange(
                        "p a b -> p (a b)"
                    ).rearrange("p (c d) -> p c d", c=2)
                    psrk = ap.tile([128, 2, 4, 64], F32, tag="pso", bufs=1,
                                   name="psrk3").rearrange(
                        "p a b c -> p (a b c)"
                    )
                    emit_rms(so, psrq, psrk, ab)

                # (half, ch) -> kn packing with that kv head at the right base
                def knt_for(half, ch):
                    return kn if (ch == 0) == (half == 0) else kn_sw

                def scores_exp(g):
                    ic, jc = divmod(g, NJC)
                    isl = slice(ic * ICS, (ic + 1) * ICS)
                    pss = ap.tile([128, 4, ICS], F32, tag="pss", bufs=3,
                                  name="pss")
                    for hs in range(4):
                        half, ch = divmod(hs, 2)
                        rows = slice(half * 64, half * 64 + 64)
                        nc.tensor.matmul(
                            pss[:, hs, :],
                            lhsT=knt_for(half, ch)[rows,
                                                   jc * 128:(jc + 1) * 128],
                            rhs=qn[rows, ch, isl],
                            start=(ch == 0), stop=(ch == 1),
                        )
                    pt = ab.tile([128, 4, ICS], BF16, tag="pT", bufs=LK + 2,
                                 name="pt")
                    if use_mask:
                        mkt = ab.tile([128, ICS], F32, tag="mkt", bufs=2,
                                      name="mkt")
                        nc.sync.dma_start(out=mkt[:], in_=mk[jc][:, isl])
                        sm = ab.tile([128, 4, ICS], F32, tag="sm", bufs=2,
                                     name="sm")
                        nc.vector.scalar_tensor_tensor(
                            sm, pss[:], 0.125,
                            mkt[:, None, :].to_broadcast((128, 4, ICS)),
                            AL.mult, AL.add,
                        )
                        nc.scalar.activation(pt, sm, AF.Exp)
                    elif JC_ENG[jc] == "act":
                        nc.scalar.activation(pt, pss, AF.Exp, scale=0.125)
                    else:
                        nc.vector.tensor_scalar(
                            pt.bitcast(I16), pss[:], SCH_A, SCH_B,
                            AL.mult, AL.add,
                        )
                    pT[g] = pt

                def emit_finish(ic):
                    """reciprocal + per-isub normalize + XBAR transposes
                    (per-isub so the first transposes fire early)."""
                    po, de = pso.pop(ic)
                    rcp = ab.tile([128, 2, 4, 1], F32, tag="rcp", bufs=2,
                                  name="rcp")
                    nc.vector.reciprocal(rcp[:], de[:, :, :, 0:1])
                    on = ab.tile([128, 2, 4, 64], BF16, tag="onat", bufs=2,
                                 name="on")
                    for isub in range(2):
                        nc.vector.tensor_tensor(
                            on[:, isub], po[:, isub],
                            rcp[:, isub].to_broadcast((128, 4, 64)),
                            AL.mult,
                        )
                        for cc in range(2):
                            nc.sync.dma_start_transpose(
                                oT[:, cc, ic,
                                   isub * 128:(isub + 1) * 128],
                                on[:, isub,
                                   2 * cc:2 * cc + 2, :].rearrange(
                                    "p a b -> p (a b)"
                                ),
                            )

                def emit_tail(ic):
                    """drain: PE transposes via identity (no XBAR DMA
                    latency), out_proj pieces interleaved per isub, piece
                    copies on ACT so the DVE chain stays short."""
                    po, de = pso.pop(ic)
                    rcp = ab.tile([128, 2, 4, 1], F32, tag="rcp", bufs=2,
                                  name="rcp")
                    nc.vector.reciprocal(rcp[:], de[:, :, :, 0:1])
                    on = ab.tile([128, 2, 4, 64], BF16, tag="onat", bufs=2,
                                 name="on")
                    psT = ap.tile([128, 2, 4, 64], F32, tag="den", bufs=1,
                                  name="psT")
                    psTf = psT.rearrange("p a b c -> p (a b c)")
                    for isub in range(2):
                        nc.vector.tensor_tensor(
                            on[:, isub], po[:, isub],
                            rcp[:, isub].to_broadcast((128, 4, 64)),
                            AL.mult,
                        )
                        for cc in range(2):
                            k = isub * 2 + cc
                            nc.tensor.matmul(
                                psTf[:, k * 128:(k + 1) * 128],
                                lhsT=on[:, isub,
                                        2 * cc:2 * cc + 2, :].rearrange(
                                    "p a b -> p (a b)"),
                                rhs=ident_sb[:],
                                start=(k == 0), stop=(k == 3),
                            )
                        nc.vector.tensor_copy(
                            oT[:, :, ic, isub * 128:(isub + 1) * 128],
                            psT[:, isub, :, :].rearrange(
                                "p c d -> p (c d)"
                            ).rearrange("p (c d) -> p c d", c=2),
                        )
                        for ec in range(2):
                            outproj_piece(ic, isub * 2 + ec, tail=True)

                def pv(r):
                    ric, rjc = divmod(r, NJC)
                    po, de = pso[ric]
                    pt = pT.pop(r)
                    for isub in range(2):
                        for hs in range(4):
                            first = rjc == 0 and isub == 0 and hs == 0
                            last = rjc == NJC - 1 and isub == 1 and hs == 3
                            nc.tensor.matmul(
                                po[:, isub, hs, :],
                                lhsT=pt[:, hs, isub * 128:(isub + 1) * 128],
                                rhs=v_all[:, rjc, hs % 2, 0:64],
                                start=first, stop=last,
                            )
                            nc.tensor.matmul(
                                de[:, isub, hs, 0:1],
                                lhsT=pt[:, hs, isub * 128:(isub + 1) * 128],
                                rhs=v_all[:, rjc, hs % 2, 64:65],
                                start=first, stop=last,
                            )

                def outproj_piece(ic, k, tail=False):
                    isub, ec = divmod(k, 2)
                    psy = ap.tile([128, 4, ICS], F32, tag="pss", bufs=3,
                                  name="psy")
                    out = psy[:, 0:2, :].rearrange("p a b -> p (a b)")
                    for cc in range(2):
                        nc.tensor.matmul(
                            out,
                            lhsT=oT[:, cc, ic, isub * 128:(isub + 1) * 128],
                            rhs=wo_sb[:, cc, ec * 512:(ec + 1) * 512],
                            start=(cc == 0), stop=(cc == 1),
                        )
                    dst = y_sb[:, ic, isub, ec * 512:(ec + 1) * 512]
                    if tail:
                        # drain: copies on ACT, per-ec DMAs so the final
                        # exposed chain is one copy + a small DMA
                        nc.scalar.copy(dst, out)
                        nc.sync.dma_start(
                            out=y[:, ic, isub, ec * 512:(ec + 1) * 512],
                            in_=dst,
                        )
                    else:
                        nc.vector.tensor_copy(dst, out)
                        if ec == 1:
                            nc.sync.dma_start(
                                out=y[:, ic, isub, :], in_=y_sb[:, ic, isub, :]
                            )

                # preamble: V groups 2/3 + the last rms chain ride in the
                # pss/pso rotations, absorbing the front-psum handoff;
                # exp table warm goes after the last front Sqrt so the ACT
                # function set switches exactly once
                for g23 in (2, 3):
                    psvt = ap.tile([128, 4, ICS], F32, tag="pss", bufs=3,
                                   name="psvt")
                    psv_group(g23, psvt, one_bank=False)
                emit_psr_attn(3)
                warm = cp.tile([1, 1], F32, name="warm")
                nc.scalar.activation(warm[:], eps_sb[0:1, :], AF.Exp)

                PIECE_JC = {8: 0, 10: 1, 12: 2, 15: 3}
                NG = NIC * NJC
                for g in range(NG + LK):
                    r = g - LK
                    if r >= 0 and r % NJC == 0:
                        ric = r // NJC
                        if ric >= 1:
                            emit_finish(ric - 1)
                        pso[ric] = (
                            ap.tile([128, 2, 4, 64], F32, tag="pso",
                                    bufs=1, name="pso"),
                            ap.tile([128, 2, 4, 64], F32, tag="den",
                                    bufs=1, name="den"),
                        )
                    if g < NG:
                        scores_exp(g)
                        ic, jc = divmod(g, NJC)
                        if ic >= 1 and jc in PIECE_JC:
                            outproj_piece(ic - 1, PIECE_JC[jc])
                    if r >= 0:
                        pv(r)
                emit_tail(NIC - 1)

    nc.compile()
    return nc


def _get(use_mask: bool, use_bias: bool):
    key = (use_mask, use_bias)
    if key not in _cache:
        _cache[key] = _build(use_mask, use_bias)
    return _cache[key]


def _host_prep(hidden_state, attention_mask, Wq, bq, Wk, bk, Wv, bv, Wo,
               use_mask, use_bias):
    half_q, half_k = HID // 2, (HKV * D) // 2  # 512, 128
    inv_q = ROPE_BASE ** (-np.arange(half_q, dtype=np.float64) / half_q)
    inv_k = ROPE_BASE ** (-np.arange(half_k, dtype=np.float64) / half_k)
    s_idx = np.arange(S, dtype=np.float64)
    cos_q = np.cos(inv_q[:, None] * s_idx[None, :])  # [512, S]
    sin_q = np.sin(inv_q[:, None] * s_idx[None, :])
    cos_k = np.cos(inv_k[:, None] * s_idx[None, :])  # [128, S]
    sin_k = np.sin(inv_k[:, None] * s_idx[None, :])

    in_maps = []
    for core in range(8):
        b, s = core // 4, core % 4
        qA = np.arange(128 * s, 128 * s + 128)       # chA q cols
        qB = qA + 512                                 # chB q cols
        kva = s // 2
        kA = np.arange(64 * kva, 64 * kva + 64)       # kv_a cols
        kB = kA + 128                                 # kv_b cols

        # hT layout: [hidden-dim-within-chunk, ko-chunk, S]
        hTc = np.ascontiguousarray(
            hidden_state[b].T.reshape(8, 128, S).transpose(1, 0, 2)
        ).astype(NB)
        wq_c = np.stack(
            [Wq[:, np.concatenate([qA, qB])][ko * 128:(ko + 1) * 128]
             for ko in range(8)], axis=1,
        ).astype(NB)  # [128, 8, 256]
        wk_cols = np.concatenate([Wk[:, kA], Wk[:, kB]], axis=1)  # [HID, 128]
        wv_cols = np.concatenate([Wv[:, kA], Wv[:, kB]], axis=1)  # [HID, 128]
        wkv_c = np.stack(
            [np.concatenate([wk_cols, wv_cols], axis=1)[ko * 128:(ko + 1) * 128]
             for ko in range(8)], axis=1,
        ).astype(NB)  # [128, 8, 256]
        worows = np.concatenate([qA[0:64], qB[0:64], qA[64:128], qB[64:128]])
        wo_c = Wo[worows].astype(NB).reshape(2, 128, HID).transpose(1, 0, 2)
        wo_c = np.ascontiguousarray(wo_c)
        qtab_c = np.stack(
            [cos_q[qA % 512], -sin_q[qA % 512], sin_q[qA % 512]], axis=1
        ).astype(NB)  # [128, 3, S]: cos, -sin, +sin
        # packed-K rope tables: cos duplicated to both partition halves;
        # sin with crossed signs (+sin on 0:64 feeds the upper-half output,
        # -sin on 64:128 feeds the lower-half output)
        kfreq = kA % 128
        ktab_c = np.stack(
            [np.concatenate([cos_k[kfreq], cos_k[kfreq]], axis=0),
             np.concatenate([sin_k[kfreq], -sin_k[kfreq]], axis=0)], axis=1
        ).astype(NB)  # [128, 2, S]

        m = {
            "hT": hTc, "wq": wq_c, "wkv": wkv_c, "wo": wo_c,
            "qtab": qtab_c, "ktab": ktab_c,
            "ident": np.eye(128, dtype=np.float32).astype(NB),
        }
        if use_bias:
            m["bias"] = np.concatenate(
                [bq[qA], bq[qB], bk[kA], bk[kB], bv[kA], bv[kB]]
            ).astype(np.float32).reshape(1, 512)
        if use_mask:
            mT = np.ascontiguousarray(attention_mask[b].T).astype(np.float32)
            m["mk"] = mT.reshape(NJC, 128, S)
        in_maps.append(m)
    return in_maps


def kernel(hidden_state, attention_mask, Wq, bq, Wk, bk, Wv, bv, Wo, bo):
    hidden_state = np.asarray(hidden_state, dtype=np.float32)
    attention_mask = np.asarray(attention_mask, dtype=np.float32)
    bq, bk, bv = (np.asarray(x, np.float32) for x in (bq, bk, bv))
    use_mask = bool(np.any(attention_mask))
    use_bias = bool(np.any(bq) or np.any(bk) or np.any(bv))
    nc = _get(use_mask, use_bias)
    in_maps = _host_prep(
        hidden_state, attention_mask,
        np.asarray(Wq, np.float32), bq,
        np.asarray(Wk, np.float32), bk,
        np.asarray(Wv, np.float32), bv,
        np.asarray(Wo, np.float32), use_mask, use_bias,
    )
    res = run_bass_kernel_spmd(nc, in_maps, list(range(8)))
    out = np.zeros((B, S, HID), dtype=np.float32)
    for core in range(8):
        yc = res.results[core]["y"].astype(np.float32)  # [128, NIC, 2, HID]
        out[core // 4] += yc.transpose(1, 2, 0, 3).reshape(S, HID)
    out += np.asarray(bo, np.float32)[None, None, :]
    return out
